# revision 26
# baseline (speedup 1.0000x reference)
"""DisMaxLossFirstPart forward on 8 Trainium2 NeuronCores.

logits = -(iso + mean_c(iso)) / temperature
  iso   = |distance_scale| * sqrt(max(2 - 2*cos(f_b, p_c), 0)) / sqrt(2)
        = sqrt(ds^2 * max(1 - cos(f_b, p_c), 0))

Data-parallel: batch (16384) sharded 8 ways across the cores; prototypes
replicated; no collectives (the per-row mean is local).

v3 design (99.4us bf16 baseline -> 81.5us v2 -> this):
  - everything fp8(e4m3): features ship both natural (row norms) and
    host-pre-transposed as fT [128, 8k, 2048b]; prototypes ship fp8
    natural.  Total DMA 9 MiB/core (14 in the baseline).
  - main matmul: 2 c-chunks x 4 DoubleRow MMs (K=256) per 128-row block
    into one fused 2-bank [128, 1000] psum tile.
  - prototype preamble fuses the normalize INTO the PE transpose:
    rhs = ident * (-S_p/||p||) in bf16 (mixed fp8 x bf16 matmul, verified
    on HW), psum->sbuf copy quantizes p_hat to fp8.  The p row norms run
    on the ACT (Square+accum) during its otherwise idle preamble window.
  - f row norms: DVE scalar_tensor_tensor fp8 square with fused row-sum
    (1213ns measured vs 2899ns for bf16); ||f8|| is exactly the norm of
    the fp8 values used in the matmul.
  - iso = Sqrt(scl_b * psum + ds^2) in ONE ACT pass per block with
    accum_out row sums; logits = (-1/T)*iso + m on DVE/GPSIMD (bf16 4x).
  - output bf16, upcast on host (|logits|~2, tol 2e-2).
  - 8 dummy warm matmuls at t=0 (HAM clock-gate warm through the DMA
    window); all preamble memsets on vector/scalar engines (GPSIMD's
    first ucode call is expensive).

distance_scale / temperature are [1]-element runtime inputs baked into
the program as immediates (rebuilt per call; correct for any values at
the cost of a recompile).
"""

import os

import numpy as np

N_CORES = 8
B, F, C = 16384, 1024, 1000
BS = B // N_CORES          # 2048 rows per core
NB = BS // 128             # 16 feature blocks per core
KT = F // 128              # 8 contraction chunks (paired 2x for DoubleRow)
KC = KT // 2               # 4 DoubleRow chunks of K=256
CHUNKS = ((0, 512), (512, 488))   # c-chunks, bank-aligned halves of psum
PJ = (C + 127) // 128      # 8 prototype row-tiles (last one 104 rows)
CPAD = 1024                # padded c-plane stride for pT (16B-aligned)
S_P = 16.0                 # fp8 scale on normalized prototypes


def _build_program(ds2: float, neg_inv_t: float):
    from contextlib import ExitStack

    import concourse.tile as tile
    from concourse import bacc, mybir
    from concourse.masks import make_identity

    f32 = mybir.dt.float32
    bf16 = mybir.dt.bfloat16
    fp8 = mybir.dt.float8e4
    AF = mybir.ActivationFunctionType
    ALU = mybir.AluOpType
    DR = mybir.MatmulPerfMode.DoubleRow

    # nrm = Sqrt(ss * S_p^2/ds^4) = S_p*||f||/ds^2 ; scl = 1/nrm
    inv_ds4 = (S_P * S_P) / (ds2 * ds2)

    nc = bacc.Bacc("TRN2", target_bir_lowering=False, debug=False,
                   num_devices=N_CORES)

    fTdr = nc.dram_tensor("fT", [128, KT, BS], fp8, kind="ExternalInput").ap()
    pdr = nc.dram_tensor("prototypes", [C, F], fp8,
                         kind="ExternalInput").ap()
    odr = nc.dram_tensor("out", [BS, C], bf16, kind="ExternalOutput").ap()

    with tile.TileContext(nc) as tc, ExitStack() as ctx:
        const_pool = ctx.enter_context(tc.tile_pool(name="const", bufs=1))
        ident = const_pool.tile([128, 128], bf16, tag="ident")
        make_identity(nc, ident[:])
        bias_ds2 = const_pool.tile([128, 1], f32, tag="bias_ds2")
        nc.vector.memset(bias_ds2[:], ds2)
        identf = const_pool.tile([128, 128], f32, tag="identf")
        nc.vector.tensor_copy(identf[:], ident[:])
        # ACT warmup: pull the Sqrt table set in during the DMA window
        warm = const_pool.tile([128, 1], f32, tag="warm")
        nc.scalar.activation(warm[:], bias_ds2[:], AF.Sqrt)
        # PE warmup operands (vector/scalar memsets only - no GPSIMD here)
        wl = const_pool.tile([128, 16], bf16, tag="wl")
        nc.vector.memset(wl[:], 0.0)
        wr = const_pool.tile([128, 512], bf16, tag="wr")
        nc.vector.memset(wr[:], 0.0)

        # persistent fp8 operands for the main matmul
        fT_pool = ctx.enter_context(tc.tile_pool(name="fT", bufs=1))
        fT8 = fT_pool.tile([128, KT, BS], fp8, tag="fT8", name="fT8")
        pT_pool = ctx.enter_context(tc.tile_pool(name="pT", bufs=1))
        pT8 = pT_pool.tile([128, KT, CPAD], fp8, tag="pT8", name="pT8")

        # PSUM: spsum 3 x [128,1024] = banks 0-5; ppsum 2 x [128,512] = 6-7
        spsum = ctx.enter_context(tc.tile_pool(name="spsum", bufs=3,
                                               space="PSUM"))
        ppsum = ctx.enter_context(tc.tile_pool(name="ppsum", bufs=2,
                                               space="PSUM"))
        # dummy matmuls keep the PE busy (HAM warm) during initial DMA;
        # they write into the preamble psum tiles (reused right after).
        def warm_mms(n, tag):
            for wi in range(n):
                wt = ppsum.tile([128, 512], f32, tag="ptps",
                                name=f"warm{tag}{wi}")
                nc.tensor.matmul(wt[:16, :], lhsT=wl[:], rhs=wr[:],
                                 start=True, stop=True)

        warm_mms(10, "a")

        # ---- input DMAs in landing-priority order -------------------------
        # praw j0-3 -> fT kc0 -> praw j4-7 -> fT kc1-3 (fnat streams in prep)
        pload = ctx.enter_context(tc.tile_pool(name="pload", bufs=1))
        praw = []
        for j in range(PJ):
            rows = min(128, C - j * 128)
            pr = pload.tile([128, F], fp8, tag=f"praw{j}", name=f"praw{j}")
            praw.append((pr, rows))

        def dma_praw(j):
            nc.sync.dma_start(out=praw[j][0][:praw[j][1]],
                              in_=pdr[j * 128: j * 128 + praw[j][1]])

        def dma_fT(kc):
            nc.sync.dma_start(out=fT8[:, 2 * kc:2 * kc + 2, :],
                              in_=fTdr[:, 2 * kc:2 * kc + 2, :])

        for j in range(PJ):
            dma_praw(j)
        for kc in range(KC):
            dma_fT(kc)

        # ---- prototype preamble -------------------------------------------
        with tc.tile_pool(name="psmall", bufs=1) as psmall, \
             tc.tile_pool(name="psq", bufs=2) as psqp:
            sidents = [None] * PJ

            def p_chain(j):
                """Norm chain for one prototype tile -> scaled identity."""
                pr, rows = praw[j]
                sq = psqp.tile([128, F], bf16, tag="psq", name=f"psq{j}")
                ss = psmall.tile([128, 1], f32, tag=f"pss{j}")
                # alternate engines so chains run pairwise in parallel
                if j % 2 == 0:
                    nc.vector.scalar_tensor_tensor(
                        sq[:rows], pr[:rows], 1.0, pr[:rows],
                        ALU.mult, ALU.mult, accum_out=ss[:rows])
                else:
                    nc.scalar.activation(sq[:rows], pr[:rows], AF.Square,
                                         accum_out=ss[:rows])
                nrm = psmall.tile([128, 1], f32, tag=f"pnrm{j}")
                nc.scalar.activation(nrm[:rows], ss[:rows], AF.Sqrt)
                inv = psmall.tile([128, 1], f32, tag=f"pinv{j}")
                nc.vector.reciprocal(inv[:rows], nrm[:rows])
                # scaled identity: diag = -S_p/||p||
                sid = psmall.tile([128, 128], bf16, tag=f"sid{j}")
                nc.vector.tensor_scalar(sid[:rows], ident[:rows], inv[:rows],
                                        -S_P, ALU.mult, ALU.mult)
                sidents[j] = sid

            def p_transposes(jg):
                """Transpose+normalize: psum[f, c] = praw[c, f] * diag[c]."""
                for k in range(KT):
                    j0 = jg * 4
                    cols = sum(praw[j][1] for j in range(j0, j0 + 4))
                    pt_ps = ppsum.tile([128, 512], f32, tag="ptps",
                                       name=f"ptps_{jg}_{k}")
                    for jj in range(4):
                        j = j0 + jj
                        pr, rows = praw[j]
                        nc.tensor.matmul(
                            pt_ps[:, jj * 128: jj * 128 + rows],
                            lhsT=pr[:rows, k * 128:(k + 1) * 128],
                            rhs=sidents[j][:rows, :rows],
                            start=True, stop=True)
                    dst = pT8[:, k, jg * 512: jg * 512 + cols]
                    # copies alternate engines so neither FIFO serializes
                    # the full set
                    if k % 2 == 0:
                        nc.scalar.copy(dst, pt_ps[:, :cols])
                    else:
                        nc.vector.tensor_copy(dst, pt_ps[:, :cols])

            for j in range(PJ):
                p_chain(j)
            p_transposes(0)
            # bridge the sident-j4..7 wait so the HAM clock-gate stays warm
            warm_mms(3, "b")
            p_transposes(1)

        # ---- main loop over 16 feature blocks -----------------------------
        with tc.tile_pool(name="fsq", bufs=2) as fsqp, \
             tc.tile_pool(name="iso", bufs=3) as isop, \
             tc.tile_pool(name="osb", bufs=3) as osbp, \
             tc.tile_pool(name="small", bufs=6) as smallp:

            def compute(bi):
                sp = spsum.tile([128, 1024], f32, tag="spsum")
                # diag psum (reuses the preamble pool's bank rotation):
                # self-matmul Gram diagonal gives ||f8_b||^2 exactly, with
                # the same stationary weights as the main matmuls.
                dg = ppsum.tile([128, 512], f32, tag="ptps",
                                name=f"diag{bi}")
                # kc-outer: diag + both c-chunks share each LDWEIGHTS
                for kc in range(KC):
                    lhs = fT8[:, 2 * kc:2 * kc + 2, bi * 128:(bi + 1) * 128]
                    nc.tensor.matmul(dg[:, 0:128], lhsT=lhs, rhs=lhs,
                                     start=(kc == 0), stop=(kc == KC - 1),
                                     perf_mode=DR)
                    for cbase, cw in CHUNKS:
                        nc.tensor.matmul(
                            sp[:, cbase:cbase + cw],
                            lhsT=lhs,
                            rhs=pT8[:, 2 * kc:2 * kc + 2, cbase:cbase + cw],
                            start=(kc == 0), stop=(kc == KC - 1),
                            perf_mode=DR)
                # ss_b = sum_c (diag_psum * I)[b, c] = ||f8_b||^2
                sq = fsqp.tile([128, 128], f32, tag="fsqscr")
                ss = smallp.tile([128, 1], f32, tag="fss")
                nc.vector.tensor_tensor(sq[:], dg[:, 0:128], identf[:],
                                        ALU.mult)
                nc.vector.tensor_reduce(ss[:], sq[:],
                                        mybir.AxisListType.X, ALU.add)
                nrm = smallp.tile([128, 1], f32, tag="fnrm")
                nc.scalar.activation(nrm[:], ss[:], AF.Sqrt, scale=inv_ds4)
                scl = smallp.tile([128, 1], f32, tag="fscl")
                nc.vector.reciprocal(scl[:], nrm[:])
                iso = isop.tile([128, C], bf16, tag="iso")
                rs = smallp.tile([128, 1], f32, tag="rs")
                nc.scalar.activation(iso[:], sp[:, :C], AF.Sqrt,
                                     bias=bias_ds2[:], scale=scl[:],
                                     accum_out=rs[:])
                m = smallp.tile([128, 1], f32, tag="m")
                nc.vector.tensor_scalar_mul(m[:], rs[:], neg_inv_t / C)
                ob = osbp.tile([128, C], bf16, tag="osb")
                # last blocks on DVE (shorter tail chain); earlier blocks
                # alternate so GPSIMD absorbs half the mid-kernel work
                eng = nc.gpsimd if (bi % 2 == 0 and bi < 12) else nc.vector
                eng.tensor_scalar(ob[:], iso[:], neg_inv_t, m[:],
                                  ALU.mult, ALU.add)
                # out-DMA triggers ride the otherwise idle GPSIMD queue so
                # they never head-block input DMAs on the sync queue; the
                # last two ride the (by then idle) sync queue for a
                # shorter tail chain
                trig = nc.sync if bi >= NB - 2 else nc.gpsimd
                trig.dma_start(out=odr[bi * 128:(bi + 1) * 128],
                               in_=ob[:])

            for bi in range(NB):
                compute(bi)

    nc.compile()
    return nc


def kernel(features, prototypes, distance_scale, temperature):
    from concourse.bass_utils import run_bass_kernel_spmd

    import ml_dtypes

    e4 = ml_dtypes.float8_e4m3

    f8 = np.ascontiguousarray(features, dtype=np.float32).astype(e4)
    p8 = np.ascontiguousarray(prototypes, dtype=np.float32).astype(e4)

    ds2 = float(abs(float(np.asarray(distance_scale).reshape(-1)[0])) ** 2)
    neg_inv_t = -1.0 / float(np.asarray(temperature).reshape(-1)[0])

    nc = _build_program(ds2, neg_inv_t)

    in_maps = []
    for i in range(N_CORES):
        X8 = f8[i * BS:(i + 1) * BS]          # [2048, 1024]
        # [p, k, b] = f8[b, k*128 + p]
        fT = np.ascontiguousarray(
            X8.T.reshape(KT, 128, BS).transpose(1, 0, 2))
        in_maps.append({"fT": fT, "prototypes": p8})

    trace_dir = os.environ.get("KERNEL_TRACE_DIR")
    if trace_dir:
        res = run_bass_kernel_spmd(nc, in_maps, list(range(N_CORES)),
                                   trace=True, tmpdir=trace_dir)
        print(f"HW exec time: {res.exec_time_ns} ns")
        print(f"mean core exec time: {res.mean_exec_time_ns} ns")
    else:
        res = run_bass_kernel_spmd(nc, in_maps, list(range(N_CORES)))

    return np.concatenate(
        [res.results[i]["out"].astype(np.float32) for i in range(N_CORES)],
        axis=0)


# revision 27
# speedup vs baseline: 1.0250x; 1.0250x over previous
"""DisMaxLossFirstPart forward on 8 Trainium2 NeuronCores.

logits = -(iso + mean_c(iso)) / temperature
  iso   = |distance_scale| * sqrt(max(2 - 2*cos(f_b, p_c), 0)) / sqrt(2)
        = sqrt(ds^2 * max(1 - cos(f_b, p_c), 0))

Data-parallel: batch (16384) sharded 8 ways across the cores; prototypes
replicated; no collectives (the per-row mean is local).

v3 design (99.4us bf16 baseline -> 81.5us v2 -> this):
  - everything fp8(e4m3): features ship both natural (row norms) and
    host-pre-transposed as fT [128, 8k, 2048b]; prototypes ship fp8
    natural.  Total DMA 9 MiB/core (14 in the baseline).
  - main matmul: 2 c-chunks x 4 DoubleRow MMs (K=256) per 128-row block
    into one fused 2-bank [128, 1000] psum tile.
  - prototype preamble fuses the normalize INTO the PE transpose:
    rhs = ident * (-S_p/||p||) in bf16 (mixed fp8 x bf16 matmul, verified
    on HW), psum->sbuf copy quantizes p_hat to fp8.  The p row norms run
    on the ACT (Square+accum) during its otherwise idle preamble window.
  - f row norms: DVE scalar_tensor_tensor fp8 square with fused row-sum
    (1213ns measured vs 2899ns for bf16); ||f8|| is exactly the norm of
    the fp8 values used in the matmul.
  - iso = Sqrt(scl_b * psum + ds^2) in ONE ACT pass per block with
    accum_out row sums; logits = (-1/T)*iso + m on DVE/GPSIMD (bf16 4x).
  - output bf16, upcast on host (|logits|~2, tol 2e-2).
  - 8 dummy warm matmuls at t=0 (HAM clock-gate warm through the DMA
    window); all preamble memsets on vector/scalar engines (GPSIMD's
    first ucode call is expensive).

distance_scale / temperature are [1]-element runtime inputs baked into
the program as immediates (rebuilt per call; correct for any values at
the cost of a recompile).
"""

import os

import numpy as np

N_CORES = 8
B, F, C = 16384, 1024, 1000
BS = B // N_CORES          # 2048 rows per core
NB = BS // 128             # 16 feature blocks per core
KT = F // 128              # 8 contraction chunks (paired 2x for DoubleRow)
KC = KT // 2               # 4 DoubleRow chunks of K=256
CHUNKS = ((0, 512), (512, 488))   # c-chunks, bank-aligned halves of psum
PJ = (C + 127) // 128      # 8 prototype row-tiles (last one 104 rows)
CPAD = 1024                # padded c-plane stride for pT (16B-aligned)
S_P = 16.0                 # fp8 scale on normalized prototypes


def _build_program(ds2: float, neg_inv_t: float):
    from contextlib import ExitStack

    import concourse.tile as tile
    from concourse import bacc, mybir
    from concourse.masks import make_identity

    f32 = mybir.dt.float32
    bf16 = mybir.dt.bfloat16
    fp8 = mybir.dt.float8e4
    AF = mybir.ActivationFunctionType
    ALU = mybir.AluOpType
    DR = mybir.MatmulPerfMode.DoubleRow

    # nrm = Sqrt(ss * S_p^2/ds^4) = S_p*||f||/ds^2 ; scl = 1/nrm
    inv_ds4 = (S_P * S_P) / (ds2 * ds2)

    nc = bacc.Bacc("TRN2", target_bir_lowering=False, debug=False,
                   num_devices=N_CORES)

    fTdr = nc.dram_tensor("fT", [128, KT, BS], fp8, kind="ExternalInput").ap()
    pdr = nc.dram_tensor("prototypes", [C, F], fp8,
                         kind="ExternalInput").ap()
    odr = nc.dram_tensor("out", [BS, C], bf16, kind="ExternalOutput").ap()

    with tile.TileContext(nc) as tc, ExitStack() as ctx:
        const_pool = ctx.enter_context(tc.tile_pool(name="const", bufs=1))
        ident = const_pool.tile([128, 128], bf16, tag="ident")
        make_identity(nc, ident[:])
        bias_ds2 = const_pool.tile([128, 1], f32, tag="bias_ds2")
        nc.vector.memset(bias_ds2[:], ds2)
        identf = const_pool.tile([128, 128], f32, tag="identf")
        nc.vector.tensor_copy(identf[:], ident[:])
        # ACT warmup: pull the Sqrt table set in during the DMA window
        warm = const_pool.tile([128, 1], f32, tag="warm")
        nc.scalar.activation(warm[:], bias_ds2[:], AF.Sqrt)
        # PE warmup operands (vector/scalar memsets only - no GPSIMD here)
        wl = const_pool.tile([128, 16], bf16, tag="wl")
        nc.vector.memset(wl[:], 0.0)
        wr = const_pool.tile([128, 512], bf16, tag="wr")
        nc.vector.memset(wr[:], 0.0)

        # persistent fp8 operands for the main matmul
        fT_pool = ctx.enter_context(tc.tile_pool(name="fT", bufs=1))
        fT8 = fT_pool.tile([128, KT, BS], fp8, tag="fT8", name="fT8")
        pT_pool = ctx.enter_context(tc.tile_pool(name="pT", bufs=1))
        pT8 = pT_pool.tile([128, KT, CPAD], fp8, tag="pT8", name="pT8")

        # PSUM: spsum 2 x [128,1024] = banks 0-3; ppsum 4 x [128,512] = 4-7
        # (4 preamble bufs so the transposes run ahead of the psum->sbuf
        # copies instead of stalling on copy backpressure; the per-block
        # Gram-diagonal psums also rotate through this pool)
        spsum = ctx.enter_context(tc.tile_pool(name="spsum", bufs=2,
                                               space="PSUM"))
        ppsum = ctx.enter_context(tc.tile_pool(name="ppsum", bufs=4,
                                               space="PSUM"))
        # dummy matmuls keep the PE busy (HAM warm) during initial DMA;
        # they write into the preamble psum tiles (reused right after).
        def warm_mms(n, tag):
            for wi in range(n):
                wt = ppsum.tile([128, 512], f32, tag="ptps",
                                name=f"warm{tag}{wi}")
                nc.tensor.matmul(wt[:16, :], lhsT=wl[:], rhs=wr[:],
                                 start=True, stop=True)

        warm_mms(10, "a")

        # ---- input DMAs in landing-priority order -------------------------
        # praw j0-3 -> fT kc0 -> praw j4-7 -> fT kc1-3 (fnat streams in prep)
        pload = ctx.enter_context(tc.tile_pool(name="pload", bufs=1))
        praw = []
        for j in range(PJ):
            rows = min(128, C - j * 128)
            pr = pload.tile([128, F], fp8, tag=f"praw{j}", name=f"praw{j}")
            praw.append((pr, rows))

        def dma_praw(j):
            nc.sync.dma_start(out=praw[j][0][:praw[j][1]],
                              in_=pdr[j * 128: j * 128 + praw[j][1]])

        def dma_fT(kc):
            nc.sync.dma_start(out=fT8[:, 2 * kc:2 * kc + 2, :],
                              in_=fTdr[:, 2 * kc:2 * kc + 2, :])

        for j in range(PJ):
            dma_praw(j)
        for kc in range(KC):
            dma_fT(kc)

        # ---- prototype preamble -------------------------------------------
        with tc.tile_pool(name="psmall", bufs=1) as psmall, \
             tc.tile_pool(name="psq", bufs=2) as psqp:
            sidents = [None] * PJ

            def p_chain(j):
                """Norm chain for one prototype tile -> scaled identity."""
                pr, rows = praw[j]
                sq = psqp.tile([128, F], bf16, tag="psq", name=f"psq{j}")
                ss = psmall.tile([128, 1], f32, tag=f"pss{j}")
                # alternate engines so chains run pairwise in parallel
                if j % 2 == 0:
                    nc.vector.scalar_tensor_tensor(
                        sq[:rows], pr[:rows], 1.0, pr[:rows],
                        ALU.mult, ALU.mult, accum_out=ss[:rows])
                else:
                    nc.scalar.activation(sq[:rows], pr[:rows], AF.Square,
                                         accum_out=ss[:rows])
                nrm = psmall.tile([128, 1], f32, tag=f"pnrm{j}")
                nc.scalar.activation(nrm[:rows], ss[:rows], AF.Sqrt)
                inv = psmall.tile([128, 1], f32, tag=f"pinv{j}")
                nc.vector.reciprocal(inv[:rows], nrm[:rows])
                # scaled identity: diag = -S_p/||p||
                sid = psmall.tile([128, 128], bf16, tag=f"sid{j}")
                nc.vector.tensor_scalar(sid[:rows], ident[:rows], inv[:rows],
                                        -S_P, ALU.mult, ALU.mult)
                sidents[j] = sid

            def p_transposes(jg):
                """Transpose+normalize: psum[f, c] = praw[c, f] * diag[c]."""
                for k in range(KT):
                    j0 = jg * 4
                    cols = sum(praw[j][1] for j in range(j0, j0 + 4))
                    pt_ps = ppsum.tile([128, 512], f32, tag="ptps",
                                       name=f"ptps_{jg}_{k}")
                    for jj in range(4):
                        j = j0 + jj
                        pr, rows = praw[j]
                        nc.tensor.matmul(
                            pt_ps[:, jj * 128: jj * 128 + rows],
                            lhsT=pr[:rows, k * 128:(k + 1) * 128],
                            rhs=sidents[j][:rows, :rows],
                            start=True, stop=True)
                    dst = pT8[:, k, jg * 512: jg * 512 + cols]
                    # copies alternate engines so neither FIFO serializes
                    # the full set
                    if k % 2 == 0:
                        nc.scalar.copy(dst, pt_ps[:, :cols])
                    else:
                        nc.vector.tensor_copy(dst, pt_ps[:, :cols])

            for j in range(PJ):
                p_chain(j)
            p_transposes(0)
            # bridge the sident-j4..7 wait so the HAM clock-gate stays warm
            warm_mms(3, "b")
            p_transposes(1)

        # ---- main loop over 16 feature blocks -----------------------------
        with tc.tile_pool(name="fsq", bufs=2) as fsqp, \
             tc.tile_pool(name="iso", bufs=3) as isop, \
             tc.tile_pool(name="osb", bufs=3) as osbp, \
             tc.tile_pool(name="small", bufs=6) as smallp:

            def compute(bi):
                sp = spsum.tile([128, 1024], f32, tag="spsum")
                # diag psum (reuses the preamble pool's bank rotation):
                # self-matmul Gram diagonal gives ||f8_b||^2 exactly, with
                # the same stationary weights as the main matmuls.
                dg = ppsum.tile([128, 512], f32, tag="ptps",
                                name=f"diag{bi}")
                # kc-outer: diag + both c-chunks share each LDWEIGHTS
                for kc in range(KC):
                    lhs = fT8[:, 2 * kc:2 * kc + 2, bi * 128:(bi + 1) * 128]
                    nc.tensor.matmul(dg[:, 0:128], lhsT=lhs, rhs=lhs,
                                     start=(kc == 0), stop=(kc == KC - 1),
                                     perf_mode=DR)
                    for cbase, cw in CHUNKS:
                        nc.tensor.matmul(
                            sp[:, cbase:cbase + cw],
                            lhsT=lhs,
                            rhs=pT8[:, 2 * kc:2 * kc + 2, cbase:cbase + cw],
                            start=(kc == 0), stop=(kc == KC - 1),
                            perf_mode=DR)
                # ss_b = sum_c (diag_psum * I)[b, c] = ||f8_b||^2
                sq = fsqp.tile([128, 128], f32, tag="fsqscr")
                ss = smallp.tile([128, 1], f32, tag="fss")
                nc.vector.tensor_tensor(sq[:], dg[:, 0:128], identf[:],
                                        ALU.mult)
                nc.vector.tensor_reduce(ss[:], sq[:],
                                        mybir.AxisListType.X, ALU.add)
                nrm = smallp.tile([128, 1], f32, tag="fnrm")
                nc.scalar.activation(nrm[:], ss[:], AF.Sqrt, scale=inv_ds4)
                scl = smallp.tile([128, 1], f32, tag="fscl")
                nc.vector.reciprocal(scl[:], nrm[:])
                iso = isop.tile([128, C], bf16, tag="iso")
                rs = smallp.tile([128, 1], f32, tag="rs")
                nc.scalar.activation(iso[:], sp[:, :C], AF.Sqrt,
                                     bias=bias_ds2[:], scale=scl[:],
                                     accum_out=rs[:])
                m = smallp.tile([128, 1], f32, tag="m")
                nc.vector.tensor_scalar_mul(m[:], rs[:], neg_inv_t / C)
                ob = osbp.tile([128, C], bf16, tag="osb")
                # last blocks on DVE (shorter tail chain); earlier blocks
                # alternate so GPSIMD absorbs half the mid-kernel work
                eng = nc.gpsimd if (bi % 2 == 0 and bi < 12) else nc.vector
                eng.tensor_scalar(ob[:], iso[:], neg_inv_t, m[:],
                                  ALU.mult, ALU.add)
                # out-DMA triggers ride the otherwise idle GPSIMD queue so
                # they never head-block input DMAs on the sync queue; the
                # last two ride the (by then idle) sync queue for a
                # shorter tail chain
                trig = nc.sync if bi >= NB - 2 else nc.gpsimd
                trig.dma_start(out=odr[bi * 128:(bi + 1) * 128],
                               in_=ob[:])

            for bi in range(NB):
                compute(bi)

    nc.compile()
    return nc


def kernel(features, prototypes, distance_scale, temperature):
    from concourse.bass_utils import run_bass_kernel_spmd

    import ml_dtypes

    e4 = ml_dtypes.float8_e4m3

    f8 = np.ascontiguousarray(features, dtype=np.float32).astype(e4)
    p8 = np.ascontiguousarray(prototypes, dtype=np.float32).astype(e4)

    ds2 = float(abs(float(np.asarray(distance_scale).reshape(-1)[0])) ** 2)
    neg_inv_t = -1.0 / float(np.asarray(temperature).reshape(-1)[0])

    nc = _build_program(ds2, neg_inv_t)

    in_maps = []
    for i in range(N_CORES):
        X8 = f8[i * BS:(i + 1) * BS]          # [2048, 1024]
        # [p, k, b] = f8[b, k*128 + p]
        fT = np.ascontiguousarray(
            X8.T.reshape(KT, 128, BS).transpose(1, 0, 2))
        in_maps.append({"fT": fT, "prototypes": p8})

    trace_dir = os.environ.get("KERNEL_TRACE_DIR")
    if trace_dir:
        res = run_bass_kernel_spmd(nc, in_maps, list(range(N_CORES)),
                                   trace=True, tmpdir=trace_dir)
        print(f"HW exec time: {res.exec_time_ns} ns")
        print(f"mean core exec time: {res.mean_exec_time_ns} ns")
    else:
        res = run_bass_kernel_spmd(nc, in_maps, list(range(N_CORES)))

    return np.concatenate(
        [res.results[i]["out"].astype(np.float32) for i in range(N_CORES)],
        axis=0)
